# revision 1
# baseline (speedup 1.0000x reference)
"""Two-layer single-head GAT on Trainium2 (8 NeuronCores, Bass/Tile).

Strategy (graph-parallel over dst nodes):
  - Relabel nodes into "slots": 8 cores x NW windows x 128 slots. Nodes are
    assigned to cores balanced by degree (serpentine over degree-sorted
    order), then packed into windows (<=128 nodes, capped total in-degree,
    capped per-src-bucket in-degree).
  - Per layer, each core builds its shard of a node table
    row[n] = [h(64) bf16 | 1.0 | a_s_hi | a_s_lo] via matmuls (h = x@W,
    a_s = x@(W@att_src)), then the 8 shards are AllGathered so every core
    holds the full table in its DRAM.
  - Edges live on the core that owns their dst. Per-edge rows h[src] are
    fetched with dma_gather (int16 idx => the table is read in 4 bucket
    slices of <=32768 rows; bucket(src) = src_core//2).
  - One-hot scatter masks M_T[e, d, t] = (d == dst_local[e,t]) are built on
    DVE with a repeated-iota constant so both operands are packed bf16
    (2x DVE mode). Per-edge a_d[dst] comes from a scalar_tensor_tensor
    against a replicated a_d row, split across DVE and Pool engines.
  - ex = exp(leaky_relu(a_s+a_d)); softmax max-subtraction is skipped
    (scores are O(10), exp stays in fp32 range; alpha is identical).
  - Aggregation: per window PSUM accumulates lhsT=[ex*h | ex] (128e x 65)
    @ rhs=M_T[:, :, t] (128e x 128d) -> [65 x 128d]; per-group epilogue
    divides by the ex-sum row, adds bias (and relu between layers).
  - Layer-2 phase A (table build) is interleaved into layer-1's edge phase
    per window to shorten the serial tail before the second AllGather.
Outputs are written transposed ([64, slots]) and un-permuted on the host.
"""

import numpy as np
import ml_dtypes

BF16 = ml_dtypes.bfloat16

NCORES = 8
P = 128
D = 64
NEG_SLOPE = 0.2
EPS = 1e-16

EWCAP = 2040      # max total in-degree per window
NODECAP = 128     # max nodes per window
TPBMAX = 5        # tiles per (window, bucket); bucket in-degree cap = 128*TPBMAX
GRP = 3           # windows per gather group (CALL=GRP*TPB*128 must stay
                  # under ~2500: one dma_gather's walrus sub-DMA semaphore
                  # arithmetic overflows a 16-bit ISA field beyond that)
POOL_STT = 0      # a_d-expansion stt columns on Pool: the HW backend
                  # rejects TensorScalarPtr on Pool, so all run on DVE

_CACHE = {}


def _preprocess(x, edge_index):
    """Host-side partitioning/indexing. Returns per-core input arrays + meta."""
    N = x.shape[0]
    E = edge_index.shape[1]
    src = edge_index[0].astype(np.int64)
    dst = edge_index[1].astype(np.int64)

    deg = np.bincount(dst, minlength=N)

    # --- assign nodes to cores: serpentine over degree-sorted order ---
    order = np.argsort(-deg, kind="stable")
    core_of_node = np.empty(N, np.int32)
    pat = np.concatenate([np.arange(NCORES), np.arange(NCORES)[::-1]])
    core_of_node[order] = pat[np.arange(N) % (2 * NCORES)]

    bucket_of_node = core_of_node // 2  # 4 buckets of 2 cores each

    # per-node in-degree per src bucket
    deg_b = np.zeros((N, 4), np.int64)
    for b in range(4):
        m = bucket_of_node[src] == b
        deg_b[:, b] = np.bincount(dst[m], minlength=N)

    # --- pack windows per core ---
    bcap = P * TPBMAX
    windows = [[] for _ in range(NCORES)]  # list of lists of node ids
    for c in range(NCORES):
        nodes_c = order[core_of_node[order] == c]  # degree-sorted
        cur, cur_deg, cur_b = [], 0, np.zeros(4, np.int64)
        for n in nodes_c:
            d_n = deg[n]
            db_n = deg_b[n]
            if cur and (len(cur) >= NODECAP or cur_deg + d_n > EWCAP
                        or np.any(cur_b + db_n > bcap)):
                windows[c].append(cur)
                cur, cur_deg, cur_b = [], 0, np.zeros(4, np.int64)
            cur.append(n)
            cur_deg += d_n
            cur_b = cur_b + db_n
        if cur:
            windows[c].append(cur)

    nw_real = max(len(w) for w in windows)
    NG = -(-nw_real // GRP)
    NW = NG * GRP
    SLOTS_PC = NW * P
    NSLOT = NCORES * SLOTS_PC
    BSZ = NSLOT // 4
    assert BSZ <= 32768, f"int16 gather range exceeded: BSZ={BSZ}"

    # --- slot assignment ---
    slot_of_node = np.full(N, -1, np.int64)
    for c in range(NCORES):
        for w, wl in enumerate(windows[c]):
            base = c * SLOTS_PC + w * P
            slot_of_node[np.asarray(wl, np.int64)] = base + np.arange(len(wl))
    assert (slot_of_node >= 0).all()

    sslot = slot_of_node[src]
    dslot = slot_of_node[dst]
    ecore = (dslot // SLOTS_PC).astype(np.int32)
    ew = (dslot % SLOTS_PC) // P          # window within core
    eb = (sslot // BSZ).astype(np.int32)  # src bucket
    edloc = (dslot % P).astype(np.int32)  # dst slot within window
    esidx = (sslot % BSZ).astype(np.int64)  # idx within bucket slice

    # group edges by (core, window, bucket); order within a group is free
    key = ((ecore.astype(np.int64) * NW + ew) * 4 + eb)
    eorder = np.argsort(key, kind="stable")
    key_s = key[eorder]
    # counts per (c, w, b)
    cnt = np.bincount(key_s, minlength=NCORES * NW * 4).reshape(NCORES, NW, 4)
    tiles_used = -(-cnt.max(axis=0) // P)  # [NW, 4], same for all cores
    TPB = int(-(-cnt.max() // P))
    assert TPB <= TPBMAX, f"bucket cap violated: TPB={TPB}"
    CW = TPB * P                      # slots per (window, bucket)
    CALL = GRP * CW                   # idxs per dma_gather call
    NCOLS = 4 * GRP * TPB             # dstloc cols per group

    # per-core edge-slot tables
    gidx = np.zeros((NCORES, NG, 4, CALL), np.int16)
    dloc = np.full((NCORES, NG, 4, GRP * TPB, P), 300.0, np.float32)
    lastpos = np.zeros((NCORES, NG, 4), np.int64)

    starts = np.zeros(NCORES * NW * 4 + 1, np.int64)
    np.cumsum(np.bincount(key_s, minlength=NCORES * NW * 4), out=starts[1:])
    esidx_s = esidx[eorder]
    edloc_s = edloc[eorder]
    for c in range(NCORES):
        for w in range(NW):
            g, wl = divmod(w, GRP)
            for b in range(4):
                k = (c * NW + w) * 4 + b
                lo, hi = starts[k], starts[k + 1]
                n = hi - lo
                if n == 0:
                    continue
                off = wl * CW
                gidx[c, g, b, off:off + n] = esidx_s[lo:hi].astype(np.int16)
                tt = (np.arange(n) // P) + wl * TPB
                pp = np.arange(n) % P
                dloc[c, g, b, tt, pp] = edloc_s[lo:hi].astype(np.float32)
                lastpos[c, g, b] = max(lastpos[c, g, b], off + n)

    # wrap-16 + replicate to 128 partitions: [C, NG*4*128, CALL//16]
    g16 = gidx.reshape(NCORES, NG * 4, CALL // 16, 16).transpose(0, 1, 3, 2)
    g128 = np.tile(g16, (1, 1, 8, 1)).reshape(NCORES, NG * 4 * 128, CALL // 16)
    # dstloc: [C, 128, NG * 4*GRP*TPB]  col = g*NCOLS + b*(GRP*TPB) + t
    dl = dloc.transpose(0, 4, 1, 2, 3).reshape(NCORES, P, NG * NCOLS)
    dl = np.ascontiguousarray(dl).astype(BF16)

    # permuted, transposed x per core
    node_of_slot = np.full(NSLOT, -1, np.int64)
    node_of_slot[slot_of_node] = np.arange(N)
    xT = np.zeros((NCORES, D, SLOTS_PC), BF16)
    for c in range(NCORES):
        sl = node_of_slot[c * SLOTS_PC:(c + 1) * SLOTS_PC]
        valid = sl >= 0
        blk = np.zeros((SLOTS_PC, D), np.float32)
        blk[valid] = x[sl[valid]]
        xT[c] = blk.T.astype(BF16)

    meta = dict(NW=NW, NG=NG, TPB=TPB, CW=CW, CALL=CALL, NCOLS=NCOLS,
                SLOTS_PC=SLOTS_PC, NSLOT=NSLOT, BSZ=BSZ, N=N,
                tiles_used=tuple(map(tuple, tiles_used)))
    percore = dict(xT=xT, gidx=g128, dstloc=dl)
    return meta, percore, node_of_slot


def _build_program(meta):
    import concourse.bacc as bacc
    import concourse.tile as tile
    from concourse import mybir

    F32, BF, I16 = mybir.dt.float32, mybir.dt.bfloat16, mybir.dt.int16
    Alu = mybir.AluOpType
    Act = mybir.ActivationFunctionType

    NW, NG, TPB = meta["NW"], meta["NG"], meta["TPB"]
    tiles_used = meta["tiles_used"]
    CALL, NCOLS = meta["CALL"], meta["NCOLS"]
    SLOTS_PC, NSLOT, BSZ = meta["SLOTS_PC"], meta["NSLOT"], meta["BSZ"]
    GT = GRP * TPB
    EPW = 384  # epilogue batch width: GRP windows x 128 slots

    nc = bacc.Bacc("TRN2", target_bir_lowering=False, debug=False,
                   num_devices=NCORES)

    xT_d = nc.dram_tensor("xT", [D, SLOTS_PC], BF, kind="ExternalInput")
    gidx_d = nc.dram_tensor("gidx", [NG * 4 * 128, CALL // 16], I16,
                            kind="ExternalInput")
    dstloc_d = nc.dram_tensor("dstloc", [P, NG * NCOLS], BF,
                              kind="ExternalInput")
    w1cat_d = nc.dram_tensor("w1cat", [D, 65], BF, kind="ExternalInput")
    w2cat_d = nc.dram_tensor("w2cat", [D, 65], BF, kind="ExternalInput")
    wd1_d = nc.dram_tensor("wd1rep", [D, 128], BF, kind="ExternalInput")
    wd2_d = nc.dram_tensor("wd2rep", [D, 128], BF, kind="ExternalInput")
    b1_d = nc.dram_tensor("b1", [D, 1], F32, kind="ExternalInput")
    b2_d = nc.dram_tensor("b2", [D, 1], F32, kind="ExternalInput")
    ones1_d = nc.dram_tensor("ones1", [1, D], F32, kind="ExternalInput")
    out_d = nc.dram_tensor("out2T", [D, SLOTS_PC], F32, kind="ExternalOutput")

    shard = [nc.dram_tensor(f"shard{l}", [SLOTS_PC, 128], BF) for l in (1, 2)]
    tbl = [nc.dram_tensor(f"tbl{l}", [NSLOT, 128], BF, addr_space="Shared")
           for l in (1, 2)]

    with tile.TileContext(nc) as tc:
        import contextlib
        stack = contextlib.ExitStack()
        with stack:
            const = stack.enter_context(tc.tile_pool(name="const", bufs=1))
            small = stack.enter_context(tc.tile_pool(name="small", bufs=3))
            vp = stack.enter_context(tc.tile_pool(name="vp", bufs=3))
            mp = stack.enter_context(tc.tile_pool(name="mp", bufs=3))
            sc = stack.enter_context(tc.tile_pool(name="sc", bufs=3))
            ip = stack.enter_context(tc.tile_pool(name="ip", bufs=6))
            ds = stack.enter_context(tc.tile_pool(name="ds", bufs=2))
            ep = stack.enter_context(tc.tile_pool(name="ep", bufs=2))
            psA = stack.enter_context(tc.tile_pool(name="psA", bufs=2, space="PSUM"))
            psB = stack.enter_context(tc.tile_pool(name="psB", bufs=2, space="PSUM"))
            psC = stack.enter_context(tc.tile_pool(name="psC", bufs=2, space="PSUM"))
            psD = stack.enter_context(tc.tile_pool(name="psD", bufs=2, space="PSUM"))

            # constants
            iota_i = const.tile([P, 128], I16)
            nc.gpsimd.iota(iota_i[:], pattern=[[1, 128]], base=0,
                           channel_multiplier=0)
            iota_b = const.tile([P, 128], BF)
            nc.vector.tensor_copy(iota_b[:], iota_i[:])
            # repeated iota: col = d*GT + t -> value d  (for one-hot builds
            # with both operands packed, enabling the 2x DVE mode)
            iotar_i = const.tile([P, 128 * GT], I16)
            nc.gpsimd.iota(iotar_i[:], pattern=[[1, 128], [0, GT]], base=0,
                           channel_multiplier=0)
            iotar = const.tile([P, 128 * GT], BF)
            nc.vector.tensor_copy(iotar[:], iotar_i[:])
            ones1 = const.tile([1, D], F32)
            nc.sync.dma_start(ones1[:], ones1_d.ap()[:])
            w1cat = const.tile([D, 65], BF)
            nc.sync.dma_start(w1cat[:], w1cat_d.ap()[:])
            w2cat = const.tile([D, 65], BF)
            nc.sync.dma_start(w2cat[:], w2cat_d.ap()[:])
            wd1 = const.tile([D, 128], BF)
            nc.sync.dma_start(wd1[:], wd1_d.ap()[:])
            wd2 = const.tile([D, 128], BF)
            nc.sync.dma_start(wd2[:], wd2_d.ap()[:])
            b1 = const.tile([D, 1], F32)
            nc.sync.dma_start(b1[:], b1_d.ap()[:])
            b2 = const.tile([D, 1], F32)
            nc.sync.dma_start(b2[:], b2_d.ap()[:])

            # resident across layers
            x2T = const.tile([D, SLOTS_PC], BF)
            adrep = const.tile([P, NW * 128], BF)
            adpe_d = const.tile([P, NG * NCOLS], F32)
            scr_d = const.tile([P, 128], BF)
            nc.gpsimd.memset(adpe_d[:], 0)

            for i in range(3):
                vs0 = vp.tile([P, 4, GT, 128], BF, tag="vslab")
                nc.gpsimd.memset(vs0[:], 0)

            # tblrow pool buffers get their constant-1 column (position 64)
            # written once; later phase-A writes never touch that column.
            tbl_tiles = []
            for i in range(3):
                tr = small.tile([P, 67], BF, tag="tblrow")
                nc.gpsimd.memset(tr[:, 64:65], 1.0)
                tbl_tiles.append(tr)

            def phase_a(layer, w, lhs):
                """Table row + replicated-a_d build for window w. During the
                edge phase (layer 1 interleaved) the copies run on Activation
                (idle there); standalone phase A balances them onto DVE."""
                wcat = w1cat if layer == 0 else w2cat
                wdrep = wd1 if layer == 0 else wd2
                ps_tb = psA.tile([P, 65], F32)
                nc.tensor.matmul(ps_tb[:], lhsT=lhs, rhs=wcat[:],
                                 start=True, stop=True)
                ps_ad = psB.tile([P, 128], F32)
                nc.tensor.matmul(ps_ad[:], lhsT=wdrep[:], rhs=lhs,
                                 start=True, stop=True)
                tblrow = small.tile([P, 67], BF, tag="tblrow")
                nc.scalar.copy(tblrow[:, 0:64], ps_tb[:, 0:64])
                nc.scalar.copy(tblrow[:, 65:66], ps_tb[:, 64:65])
                # a_s_lo residual for extra precision
                nc.vector.tensor_tensor(out=tblrow[:, 66:67],
                                        in0=ps_tb[:, 64:65],
                                        in1=tblrow[:, 65:66],
                                        op=Alu.subtract)
                nc.sync.dma_start(shard[layer].ap()[w * P:(w + 1) * P, 0:67],
                                  tblrow[:])
                if layer == 0:
                    nc.vector.tensor_copy(adrep[:, w * 128:(w + 1) * 128],
                                          ps_ad[:])
                else:
                    nc.scalar.copy(adrep[:, w * 128:(w + 1) * 128],
                                   ps_ad[:])

            # ---- layer-0 phase A (x loaded in 27-window chunks) ----
            XC = 12
            for w in range(NW):
                if w % XC == 0:
                    xtc = small.tile([D, XC * P], BF, tag="xtc")
                    hi = min(NW, w + XC)
                    nc.sync.dma_start(xtc[:, 0:(hi - w) * P],
                                      xT_d.ap()[:, w * P:hi * P])
                phase_a(0, w, xtc[:, (w % XC) * P:(w % XC + 1) * P])

            iotar_v = iotar[:].rearrange("p (d t) -> p d t", t=GT)
            n_pool = (NCOLS * POOL_STT) // 20
            nd = NCOLS - n_pool

            def load_idx(g):
                """Start DMA of the gather index table for group g."""
                idxt = ip.tile([128, 4, CALL // 16], I16, tag="idxt")
                r0 = g * 4 * 128
                nc.sync.dma_start(
                    idxt[:],
                    gidx_d.ap()[r0:r0 + 4 * 128, :]
                    .rearrange("(b p) c -> p b c", b=4))
                return idxt

            def head(layer, g, idxt, dstl_all):
                """Gather launch + one-hot build for group g."""
                tbl_l = tbl[layer].ap()
                vslab = vp.tile([P, 4, GT, 128], BF, tag="vslab")
                for b in range(4):
                    nc.gpsimd.dma_gather(
                        out_ap=vslab[:, b, :, :],
                        in_ap=tbl_l[b * BSZ:(b + 1) * BSZ, :],
                        idxs_ap=idxt[:, b, :], num_idxs=CALL,
                        num_idxs_reg=CALL,
                        elem_size=128, single_packet=False)
                # one-hot masks M_T[e, b, d, t] in one 2x-DVE instruction
                mslab = mp.tile([P, 4, 128, GT], BF, tag="mslab")
                dstl_v = dstl_all[:, g * NCOLS:(g + 1) * NCOLS] \
                    .rearrange("p (b t) -> p b t", b=4)
                nc.vector.tensor_tensor(
                    out=mslab[:],
                    in0=iotar_v[:, None, :, :].to_broadcast([P, 4, 128, GT]),
                    in1=dstl_v[:, :, None, :].to_broadcast([P, 4, 128, GT]),
                    op=Alu.is_equal)
                return vslab, mslab

            def tail(layer, g, st, dstl_all):
                """Score + aggregate + epilogue for group g (inputs ready)."""
                bias = b1 if layer == 0 else b2
                vslab, mslab = st
                c0 = g * NCOLS
                as_t = sc.tile([P, NCOLS], F32, tag="as_t")
                nc.vector.tensor_tensor(
                    out=as_t[:].rearrange("p (b t) -> p b t", b=4),
                    in0=vslab[:, :, :, 65], in1=vslab[:, :, :, 66],
                    op=Alu.add)
                nc.vector.tensor_tensor(out=as_t[:],
                                        in0=as_t[:],
                                        in1=adpe_d[:, c0:c0 + NCOLS],
                                        op=Alu.add)
                lr = sc.tile([P, NCOLS], F32, tag="lr")
                nc.vector.scalar_tensor_tensor(
                    out=lr[:], in0=as_t[:], scalar=NEG_SLOPE,
                    in1=as_t[:], op0=Alu.mult, op1=Alu.max)
                ex = sc.tile([P, NCOLS], F32, tag="ex")
                nc.scalar.activation(ex[:], lr[:], Act.Exp)
                # V' in place: [h|1] * ex  (col 64 is the gathered 1.0)
                # per bucket so the first aggregation matmuls start early
                ex_v = ex[:].rearrange("p (b t) -> p b t", b=4)
                for b in range(4):
                    nc.vector.tensor_tensor(
                        out=vslab[:, b, :, 0:65], in0=vslab[:, b, :, 0:65],
                        in1=ex_v[:, b, :, None].to_broadcast([P, GT, 65]),
                        op=Alu.mult)
                # aggregation (per window) + group-batched epilogue
                aggs = ep.tile([65, EPW], F32, tag="aggs")
                for wl in range(GRP):
                    psagg = psC.tile([65, 128], F32)
                    k = 0
                    for b in range(4):
                        for t in range(TPB):
                            tcall = wl * TPB + t
                            nc.tensor.matmul(
                                psagg[:], lhsT=vslab[:, b, tcall, 0:65],
                                rhs=mslab[:, b, :, tcall],
                                start=(k == 0), stop=(k == 4 * TPB - 1))
                            k += 1
                    nc.scalar.copy(aggs[:, wl * 128:(wl + 1) * 128],
                                   psagg[:])
                den = ep.tile([1, EPW], F32, tag="den")
                nc.scalar.activation(den[:], aggs[64:65, :], Act.Copy,
                                     bias=EPS)
                nc.vector.reciprocal_approx_fast(den[:], den[:])
                ps_rec = psD.tile([D, EPW], F32)
                nc.tensor.matmul(ps_rec[:], lhsT=ones1[:], rhs=den[:],
                                 start=True, stop=True)
                tmp = ep.tile([D, EPW], F32, tag="tmp")
                nc.vector.tensor_tensor(out=tmp[:], in0=aggs[0:64, :],
                                        in1=ps_rec[:], op=Alu.mult)
                w0 = g * GRP
                if layer == 0:
                    nc.scalar.activation(x2T[:, w0 * P:(w0 + GRP) * P],
                                         tmp[:], Act.Relu, bias=bias[:, 0:1])
                    # layer-2 phase A, deferred one group so its x2T input
                    # is long done and PE never stalls at the queue head
                    if g >= 1:
                        for wl in range(GRP):
                            w = (g - 1) * GRP + wl
                            phase_a(1, w, x2T[:, w * P:(w + 1) * P])
                    if g == NG - 1:
                        for wl in range(GRP):
                            w = g * GRP + wl
                            phase_a(1, w, x2T[:, w * P:(w + 1) * P])
                else:
                    o2 = ep.tile([D, EPW], F32, tag="tmp2")
                    nc.scalar.activation(o2[:], tmp[:], Act.Identity,
                                         bias=bias[:, 0:1])
                    nc.sync.dma_start(
                        out_d.ap()[:, w0 * P:(w0 + GRP) * P], o2[:])

            def phase_b(layer):
                # The whole-layer dstloc table is loaded once; all a_d
                # expansion stt's run during the AllGather (they only need
                # adrep + dstloc), hiding the collective behind real work.
                dstl_all = ds.tile([P, NG * NCOLS], BF, tag="dstl_all")
                nc.sync.dma_start(dstl_all[:], dstloc_d.ap()[:, :])
                nc.gpsimd.collective_compute(
                    "AllGather", mybir.AluOpType.bypass,
                    replica_groups=[list(range(NCORES))],
                    ins=[shard[layer].ap()[:, :]],
                    outs=[tbl[layer].ap()[:, :]],
                )
                wb = 0.15 + layer * 1.6
                with tc.tile_wait_until(wb), tc.high_priority():
                    for g in range(NG):
                        c0 = g * NCOLS
                        for ci in range(NCOLS):
                            b, t = divmod(ci, GT)
                            w = g * GRP + t // TPB
                            if t % TPB >= tiles_used[w][b]:
                                continue  # column is pure padding
                            nc.vector.scalar_tensor_tensor(
                                out=scr_d[:],
                                in0=iota_b[:],
                                scalar=dstl_all[:, c0 + ci:c0 + ci + 1],
                                in1=adrep[:, w * 128:(w + 1) * 128],
                                op0=Alu.is_equal, op1=Alu.mult,
                                accum_out=adpe_d[:, c0 + ci:c0 + ci + 1])
                # steady loop, software-pipelined; wait-ts hints pin the
                # scheduler's per-engine order to the emission order
                ws = wb + 0.30
                dg = 0.013
                idxs = {0: load_idx(0), 1: load_idx(1)}
                with tc.tile_wait_until(ws):
                    st = head(layer, 0, idxs[0], dstl_all)
                for g in range(NG):
                    if g + 2 < NG:
                        with tc.tile_wait_until(ws + g * dg):
                            idxs[g + 2] = load_idx(g + 2)
                    if g + 1 < NG:
                        with tc.tile_wait_until(ws + (g + 1) * dg):
                            st_next = head(layer, g + 1, idxs.pop(g + 1),
                                           dstl_all)
                    with tc.tile_wait_until(ws + g * dg + 0.006):
                        tail(layer, g, st, dstl_all)
                    if g + 1 < NG:
                        st = st_next

            for layer in (0, 1):
                phase_b(layer)

    nc.compile()
    return nc


def kernel(x, edge_index, W1, att_src1, att_dst1, b1, W2, att_src2,
           att_dst2, b2):
    from concourse.bass_utils import run_bass_kernel_spmd

    x = np.asarray(x, np.float32)
    edge_index = np.asarray(edge_index)
    W1 = np.asarray(W1, np.float32)
    W2 = np.asarray(W2, np.float32)

    ek = edge_index.tobytes()
    cached = _CACHE.get("pre")
    if cached is not None and cached[0] == ek and \
            np.array_equal(cached[1], x):
        _, _, meta, percore, node_of_slot = cached
    else:
        meta, percore, node_of_slot = _preprocess(x, edge_index)
        _CACHE["pre"] = (ek, x.copy(), meta, percore, node_of_slot)
    mk = tuple(sorted(meta.items()))
    cached = _CACHE.get("prog")
    if cached is not None and cached[0] == mk:
        nc = cached[1]
    else:
        nc = _build_program(meta)
        _CACHE["prog"] = (mk, nc)
    SLOTS_PC, NSLOT, N = meta["SLOTS_PC"], meta["NSLOT"], meta["N"]

    w1cat = np.concatenate([W1, (W1 @ np.asarray(att_src1, np.float32))[:, None]],
                           axis=1).astype(BF16)
    w2cat = np.concatenate([W2, (W2 @ np.asarray(att_src2, np.float32))[:, None]],
                           axis=1).astype(BF16)
    wd1 = np.tile((W1 @ np.asarray(att_dst1, np.float32))[:, None],
                  (1, 128)).astype(BF16)
    wd2 = np.tile((W2 @ np.asarray(att_dst2, np.float32))[:, None],
                  (1, 128)).astype(BF16)
    b1c = np.asarray(b1, np.float32)[:, None]
    b2c = np.asarray(b2, np.float32)[:, None]
    ones1 = np.ones((1, D), np.float32)

    in_maps = []
    for c in range(NCORES):
        in_maps.append({
            "xT": percore["xT"][c], "gidx": percore["gidx"][c],
            "dstloc": percore["dstloc"][c],
            "w1cat": w1cat, "w2cat": w2cat, "wd1rep": wd1, "wd2rep": wd2,
            "b1": b1c, "b2": b2c, "ones1": ones1,
        })
    res = run_bass_kernel_spmd(nc, in_maps, list(range(NCORES)))

    out = np.empty((N, D), np.float32)
    for c in range(NCORES):
        blk = res.results[c]["out2T"]  # [64, SLOTS_PC]
        sl = node_of_slot[c * SLOTS_PC:(c + 1) * SLOTS_PC]
        valid = sl >= 0
        out[sl[valid]] = blk.T[valid]
    return out



# revision 7
# speedup vs baseline: 1.0492x; 1.0492x over previous
"""Two-layer single-head GAT on Trainium2 (8 NeuronCores, Bass/Tile).

Strategy (graph-parallel over dst nodes):
  - Relabel nodes into "slots": 8 cores x NW windows x 128 slots. Nodes are
    assigned to cores balanced by degree (serpentine over degree-sorted
    order), then packed into windows (<=128 nodes, capped total in-degree,
    capped per-src-bucket in-degree).
  - Per layer, each core builds its shard of a node table
    row[n] = [h(64) bf16 | 1.0 | a_s_hi | a_s_lo] via matmuls (h = x@W,
    a_s = x@(W@att_src)), then the 8 shards are AllGathered so every core
    holds the full table in its DRAM.
  - Edges live on the core that owns their dst. Per-edge rows h[src] are
    fetched with dma_gather (int16 idx => the table is read in 4 bucket
    slices of <=32768 rows; bucket(src) = src_core//2).
  - One-hot scatter masks M_T[e, d, t] = (d == dst_local[e,t]) are built on
    DVE with a repeated-iota constant so both operands are packed bf16
    (2x DVE mode). Per-edge a_d[dst] comes from a scalar_tensor_tensor
    against a replicated a_d row, split across DVE and Pool engines.
  - ex = exp(leaky_relu(a_s+a_d)); softmax max-subtraction is skipped
    (scores are O(10), exp stays in fp32 range; alpha is identical).
  - Aggregation: per window PSUM accumulates lhsT=[ex*h | ex] (128e x 65)
    @ rhs=M_T[:, :, t] (128e x 128d) -> [65 x 128d]; per-group epilogue
    divides by the ex-sum row, adds bias (and relu between layers).
  - Layer-2 phase A (table build) is interleaved into layer-1's edge phase
    per window to shorten the serial tail before the second AllGather.
Outputs are written transposed ([64, slots]) and un-permuted on the host.
"""

import numpy as np
import ml_dtypes

BF16 = ml_dtypes.bfloat16

NCORES = 8
P = 128
D = 64
NEG_SLOPE = 0.2
EPS = 1e-16

EWCAP = 2040      # max total in-degree per window
NODECAP = 128     # max nodes per window
TPBMAX = 5        # tiles per (window, bucket); bucket in-degree cap = 128*TPBMAX
GRP = 3           # windows per gather group (CALL=GRP*TPB*128 must stay
                  # under ~2500: one dma_gather's walrus sub-DMA semaphore
                  # arithmetic overflows a 16-bit ISA field beyond that)
POOL_STT = 0      # a_d-expansion stt columns on Pool: the HW backend
                  # rejects TensorScalarPtr on Pool, so all run on DVE

_CACHE = {}


def _preprocess(x, edge_index):
    """Host-side partitioning/indexing. Returns per-core input arrays + meta."""
    N = x.shape[0]
    E = edge_index.shape[1]
    src = edge_index[0].astype(np.int64)
    dst = edge_index[1].astype(np.int64)

    deg = np.bincount(dst, minlength=N)

    # --- assign nodes to cores: serpentine over degree-sorted order ---
    order = np.argsort(-deg, kind="stable")
    core_of_node = np.empty(N, np.int32)
    pat = np.concatenate([np.arange(NCORES), np.arange(NCORES)[::-1]])
    core_of_node[order] = pat[np.arange(N) % (2 * NCORES)]

    bucket_of_node = core_of_node // 2  # 4 buckets of 2 cores each

    # per-node in-degree per src bucket
    deg_b = np.zeros((N, 4), np.int64)
    for b in range(4):
        m = bucket_of_node[src] == b
        deg_b[:, b] = np.bincount(dst[m], minlength=N)

    # --- pack windows per core ---
    bcap = P * TPBMAX
    windows = [[] for _ in range(NCORES)]  # list of lists of node ids
    for c in range(NCORES):
        nodes_c = order[core_of_node[order] == c]  # degree-sorted
        cur, cur_deg, cur_b = [], 0, np.zeros(4, np.int64)
        for n in nodes_c:
            d_n = deg[n]
            db_n = deg_b[n]
            if cur and (len(cur) >= NODECAP or cur_deg + d_n > EWCAP
                        or np.any(cur_b + db_n > bcap)):
                windows[c].append(cur)
                cur, cur_deg, cur_b = [], 0, np.zeros(4, np.int64)
            cur.append(n)
            cur_deg += d_n
            cur_b = cur_b + db_n
        if cur:
            windows[c].append(cur)

    nw_real = max(len(w) for w in windows)
    NG = -(-nw_real // GRP)
    NW = NG * GRP
    SLOTS_PC = NW * P
    NSLOT = NCORES * SLOTS_PC
    BSZ = NSLOT // 4
    assert BSZ <= 32768, f"int16 gather range exceeded: BSZ={BSZ}"

    # --- slot assignment ---
    slot_of_node = np.full(N, -1, np.int64)
    for c in range(NCORES):
        for w, wl in enumerate(windows[c]):
            base = c * SLOTS_PC + w * P
            slot_of_node[np.asarray(wl, np.int64)] = base + np.arange(len(wl))
    assert (slot_of_node >= 0).all()

    sslot = slot_of_node[src]
    dslot = slot_of_node[dst]
    ecore = (dslot // SLOTS_PC).astype(np.int32)
    ew = (dslot % SLOTS_PC) // P          # window within core
    eb = (sslot // BSZ).astype(np.int32)  # src bucket
    edloc = (dslot % P).astype(np.int32)  # dst slot within window
    esidx = (sslot % BSZ).astype(np.int64)  # idx within bucket slice

    # group edges by (core, window, bucket); order within a group is free
    key = ((ecore.astype(np.int64) * NW + ew) * 4 + eb)
    eorder = np.argsort(key, kind="stable")
    key_s = key[eorder]
    # counts per (c, w, b)
    cnt = np.bincount(key_s, minlength=NCORES * NW * 4).reshape(NCORES, NW, 4)
    tiles_used = -(-cnt.max(axis=0) // P)  # [NW, 4], same for all cores
    TPB = int(-(-cnt.max() // P))
    assert TPB <= TPBMAX, f"bucket cap violated: TPB={TPB}"
    CW = TPB * P                      # slots per (window, bucket)
    CALL = GRP * CW                   # idxs per dma_gather call
    NCOLS = 4 * GRP * TPB             # dstloc cols per group

    # per-core edge-slot tables
    gidx = np.zeros((NCORES, NG, 4, CALL), np.int16)
    dloc = np.full((NCORES, NG, 4, GRP * TPB, P), 300.0, np.float32)
    lastpos = np.zeros((NCORES, NG, 4), np.int64)

    starts = np.zeros(NCORES * NW * 4 + 1, np.int64)
    np.cumsum(np.bincount(key_s, minlength=NCORES * NW * 4), out=starts[1:])
    esidx_s = esidx[eorder]
    edloc_s = edloc[eorder]
    for c in range(NCORES):
        for w in range(NW):
            g, wl = divmod(w, GRP)
            for b in range(4):
                k = (c * NW + w) * 4 + b
                lo, hi = starts[k], starts[k + 1]
                n = hi - lo
                if n == 0:
                    continue
                off = wl * CW
                gidx[c, g, b, off:off + n] = esidx_s[lo:hi].astype(np.int16)
                tt = (np.arange(n) // P) + wl * TPB
                pp = np.arange(n) % P
                dloc[c, g, b, tt, pp] = edloc_s[lo:hi].astype(np.float32)
                lastpos[c, g, b] = max(lastpos[c, g, b], off + n)

    # wrap-16 + replicate to 128 partitions: [C, NG*4*128, CALL//16]
    g16 = gidx.reshape(NCORES, NG * 4, CALL // 16, 16).transpose(0, 1, 3, 2)
    g128 = np.tile(g16, (1, 1, 8, 1)).reshape(NCORES, NG * 4 * 128, CALL // 16)
    # dstloc: [C, 128, NG * 4*GRP*TPB]  col = g*NCOLS + b*(GRP*TPB) + t
    dl = dloc.transpose(0, 4, 1, 2, 3).reshape(NCORES, P, NG * NCOLS)
    dl = np.ascontiguousarray(dl).astype(BF16)

    # permuted, transposed x per core
    node_of_slot = np.full(NSLOT, -1, np.int64)
    node_of_slot[slot_of_node] = np.arange(N)
    xT = np.zeros((NCORES, D, SLOTS_PC), BF16)
    for c in range(NCORES):
        sl = node_of_slot[c * SLOTS_PC:(c + 1) * SLOTS_PC]
        valid = sl >= 0
        blk = np.zeros((SLOTS_PC, D), np.float32)
        blk[valid] = x[sl[valid]]
        xT[c] = blk.T.astype(BF16)

    meta = dict(NW=NW, NG=NG, TPB=TPB, CW=CW, CALL=CALL, NCOLS=NCOLS,
                SLOTS_PC=SLOTS_PC, NSLOT=NSLOT, BSZ=BSZ, N=N,
                tiles_used=tuple(map(tuple, tiles_used)))
    percore = dict(xT=xT, gidx=g128, dstloc=dl)
    return meta, percore, node_of_slot


def _build_program(meta):
    import concourse.bacc as bacc
    import concourse.tile as tile
    from concourse import mybir

    F32, BF, I16 = mybir.dt.float32, mybir.dt.bfloat16, mybir.dt.int16
    Alu = mybir.AluOpType
    Act = mybir.ActivationFunctionType

    NW, NG, TPB = meta["NW"], meta["NG"], meta["TPB"]
    tiles_used = meta["tiles_used"]
    CALL, NCOLS = meta["CALL"], meta["NCOLS"]
    SLOTS_PC, NSLOT, BSZ = meta["SLOTS_PC"], meta["NSLOT"], meta["BSZ"]
    GT = GRP * TPB
    EPW = 384  # epilogue batch width: GRP windows x 128 slots

    nc = bacc.Bacc("TRN2", target_bir_lowering=False, debug=False,
                   num_devices=NCORES)

    xT_d = nc.dram_tensor("xT", [D, SLOTS_PC], BF, kind="ExternalInput")
    gidx_d = nc.dram_tensor("gidx", [NG * 4 * 128, CALL // 16], I16,
                            kind="ExternalInput")
    dstloc_d = nc.dram_tensor("dstloc", [P, NG * NCOLS], BF,
                              kind="ExternalInput")
    w1cat_d = nc.dram_tensor("w1cat", [D, 65], BF, kind="ExternalInput")
    w2cat_d = nc.dram_tensor("w2cat", [D, 65], BF, kind="ExternalInput")
    wd1_d = nc.dram_tensor("wd1rep", [D, 128], BF, kind="ExternalInput")
    wd2_d = nc.dram_tensor("wd2rep", [D, 128], BF, kind="ExternalInput")
    b1_d = nc.dram_tensor("b1", [D, 1], F32, kind="ExternalInput")
    b2_d = nc.dram_tensor("b2", [D, 1], F32, kind="ExternalInput")
    ones1_d = nc.dram_tensor("ones1", [1, D], F32, kind="ExternalInput")
    out_d = nc.dram_tensor("out2T", [D, SLOTS_PC], F32, kind="ExternalOutput")

    shard = [nc.dram_tensor(f"shard{l}", [SLOTS_PC, 128], BF) for l in (1, 2)]
    tbl = [nc.dram_tensor(f"tbl{l}", [NSLOT, 128], BF, addr_space="Shared")
           for l in (1, 2)]

    with tile.TileContext(nc) as tc:
        import contextlib
        stack = contextlib.ExitStack()
        with stack:
            const = stack.enter_context(tc.tile_pool(name="const", bufs=1))
            small = stack.enter_context(tc.tile_pool(name="small", bufs=3))
            vp = stack.enter_context(tc.tile_pool(name="vp", bufs=3))
            mp = stack.enter_context(tc.tile_pool(name="mp", bufs=3))
            sc = stack.enter_context(tc.tile_pool(name="sc", bufs=3))
            ip = stack.enter_context(tc.tile_pool(name="ip", bufs=6))
            ep = stack.enter_context(tc.tile_pool(name="ep", bufs=2))
            psA = stack.enter_context(tc.tile_pool(name="psA", bufs=2, space="PSUM"))
            psB = stack.enter_context(tc.tile_pool(name="psB", bufs=2, space="PSUM"))
            psC = stack.enter_context(tc.tile_pool(name="psC", bufs=2, space="PSUM"))
            psD = stack.enter_context(tc.tile_pool(name="psD", bufs=2, space="PSUM"))

            # constants
            iota_b = const.tile([P, 128], BF)
            iotar = const.tile([P, 128 * GT], BF)
            with tc.tile_pool(name="iotatmp", bufs=1) as iotatmp:
                iota_i = iotatmp.tile([P, 128], I16)
                nc.gpsimd.iota(iota_i[:], pattern=[[1, 128]], base=0,
                               channel_multiplier=0)
                nc.vector.tensor_copy(iota_b[:], iota_i[:])
                # repeated iota: col = d*GT + t -> value d (for one-hot
                # builds with both operands packed => 2x DVE mode)
                iotar_i = iotatmp.tile([P, 128 * GT], I16)
                nc.gpsimd.iota(iotar_i[:], pattern=[[1, 128], [0, GT]],
                               base=0, channel_multiplier=0)
                nc.vector.tensor_copy(iotar[:], iotar_i[:])
            ones1 = const.tile([1, D], F32)
            nc.sync.dma_start(ones1[:], ones1_d.ap()[:])
            w1cat = const.tile([D, 65], BF)
            nc.sync.dma_start(w1cat[:], w1cat_d.ap()[:])
            w2cat = const.tile([D, 65], BF)
            nc.sync.dma_start(w2cat[:], w2cat_d.ap()[:])
            wd1 = const.tile([D, 128], BF)
            nc.sync.dma_start(wd1[:], wd1_d.ap()[:])
            wd2 = const.tile([D, 128], BF)
            nc.sync.dma_start(wd2[:], wd2_d.ap()[:])
            b1 = const.tile([D, 1], F32)
            nc.sync.dma_start(b1[:], b1_d.ap()[:])
            b2 = const.tile([D, 1], F32)
            nc.sync.dma_start(b2[:], b2_d.ap()[:])

            # resident across layers
            x2T = const.tile([D, SLOTS_PC], BF)
            adrep = const.tile([P, NW * 128], BF)
            adpe_d = const.tile([P, NG * NCOLS], F32)
            scr_d = const.tile([P, 128], BF)
            nc.gpsimd.memset(adpe_d[:], 0)
            # whole-run dstloc table: shared by both layers, loaded once
            dstl_all = const.tile([P, NG * NCOLS], BF)
            nc.sync.dma_start(dstl_all[:], dstloc_d.ap()[:, :])

            for i in range(3):
                vs0 = vp.tile([P, 4, GT, 128], BF, tag="vslab")
                nc.gpsimd.memset(vs0[:], 0)

            # tblrow pool buffers get their constant-1 column (position 64)
            # written once; later phase-A writes never touch that column.
            tbl_tiles = []
            for i in range(3):
                tr = small.tile([P, GRP, 67], BF, tag="tblrow")
                nc.gpsimd.memset(tr[:, :, 64:65], 1.0)
                tbl_tiles.append(tr)

            def phase_a_grp(layer, w0, lhs_list):
                """Table rows + replicated-a_d for GRP consecutive windows,
                with one batched shard write. During the edge phase (layer 1
                interleaved) the copies run on Activation (idle there);
                standalone phase A balances them onto DVE."""
                wcat = w1cat if layer == 0 else w2cat
                wdrep = wd1 if layer == 0 else wd2
                tbuf = small.tile([P, GRP, 67], BF, tag="tblrow")
                for k in range(GRP):
                    lhs = lhs_list[k]
                    w = w0 + k
                    ps_tb = psA.tile([P, 65], F32)
                    nc.tensor.matmul(ps_tb[:], lhsT=lhs, rhs=wcat[:],
                                     start=True, stop=True)
                    ps_ad = psB.tile([P, 128], F32)
                    nc.tensor.matmul(ps_ad[:], lhsT=wdrep[:], rhs=lhs,
                                     start=True, stop=True)
                    if layer == 0:
                        nc.vector.tensor_copy(tbuf[:, k, 0:64],
                                              ps_tb[:, 0:64])
                        nc.vector.tensor_copy(tbuf[:, k, 65:66],
                                              ps_tb[:, 64:65])
                    else:
                        nc.scalar.copy(tbuf[:, k, 0:64], ps_tb[:, 0:64])
                        nc.scalar.copy(tbuf[:, k, 65:66], ps_tb[:, 64:65])
                    # a_s_lo residual for extra precision
                    nc.vector.tensor_tensor(out=tbuf[:, k, 66:67],
                                            in0=ps_tb[:, 64:65],
                                            in1=tbuf[:, k, 65:66],
                                            op=Alu.subtract)
                    if layer == 0:
                        nc.vector.tensor_copy(adrep[:, w * 128:(w + 1) * 128],
                                              ps_ad[:])
                    else:
                        nc.scalar.copy(adrep[:, w * 128:(w + 1) * 128],
                                       ps_ad[:])
                nc.sync.dma_start(
                    shard[layer].ap()[w0 * P:(w0 + GRP) * P, 0:67]
                    .rearrange("(k p) c -> p k c", k=GRP),
                    tbuf[:])

            # ---- layer-0 phase A (x loaded in 12-window chunks) ----
            XC = 12
            for w0 in range(0, NW, XC):
                xtc = small.tile([D, XC * P], BF, tag="xtc")
                hi = min(NW, w0 + XC)
                nc.sync.dma_start(xtc[:, 0:(hi - w0) * P],
                                  xT_d.ap()[:, w0 * P:hi * P])
                for g0 in range(w0, hi, GRP):
                    phase_a_grp(0, g0, [
                        xtc[:, (g0 + k - w0) * P:(g0 + k - w0 + 1) * P]
                        for k in range(GRP)])

            iotar_v = iotar[:].rearrange("p (d t) -> p d t", t=GT)
            n_pool = (NCOLS * POOL_STT) // 20
            nd = NCOLS - n_pool

            def load_idx(g):
                """Start DMA of the gather index table for group g."""
                idxt = ip.tile([128, 4, CALL // 16], I16, tag="idxt")
                r0 = g * 4 * 128
                nc.sync.dma_start(
                    idxt[:],
                    gidx_d.ap()[r0:r0 + 4 * 128, :]
                    .rearrange("(b p) c -> p b c", b=4))
                return idxt

            def head(layer, g, idxt, dstl_all):
                """Gather launch + one-hot build for group g."""
                tbl_l = tbl[layer].ap()
                vslab = vp.tile([P, 4, GT, 128], BF, tag="vslab")
                for b in range(4):
                    nc.gpsimd.dma_gather(
                        out_ap=vslab[:, b, :, :],
                        in_ap=tbl_l[b * BSZ:(b + 1) * BSZ, :],
                        idxs_ap=idxt[:, b, :], num_idxs=CALL,
                        num_idxs_reg=CALL,
                        elem_size=128, single_packet=False)
                # one-hot masks M_T[e, b, d, t] in one 2x-DVE instruction
                mslab = mp.tile([P, 4, 128, GT], BF, tag="mslab")
                dstl_v = dstl_all[:, g * NCOLS:(g + 1) * NCOLS] \
                    .rearrange("p (b t) -> p b t", b=4)
                nc.vector.tensor_tensor(
                    out=mslab[:],
                    in0=iotar_v[:, None, :, :].to_broadcast([P, 4, 128, GT]),
                    in1=dstl_v[:, :, None, :].to_broadcast([P, 4, 128, GT]),
                    op=Alu.is_equal)
                return vslab, mslab

            def tail(layer, g, st, dstl_all):
                """Score + aggregate + epilogue for group g (inputs ready)."""
                bias = b1 if layer == 0 else b2
                vslab, mslab = st
                c0 = g * NCOLS
                as_t = sc.tile([P, NCOLS], F32, tag="as_t")
                nc.vector.tensor_tensor(
                    out=as_t[:].rearrange("p (b t) -> p b t", b=4),
                    in0=vslab[:, :, :, 65], in1=vslab[:, :, :, 66],
                    op=Alu.add)
                nc.vector.tensor_tensor(out=as_t[:],
                                        in0=as_t[:],
                                        in1=adpe_d[:, c0:c0 + NCOLS],
                                        op=Alu.add)
                lr = sc.tile([P, NCOLS], F32, tag="lr")
                nc.vector.scalar_tensor_tensor(
                    out=lr[:], in0=as_t[:], scalar=NEG_SLOPE,
                    in1=as_t[:], op0=Alu.mult, op1=Alu.max)
                ex = sc.tile([P, NCOLS], F32, tag="ex")
                nc.scalar.activation(ex[:], lr[:], Act.Exp)
                # V' in place: [h|1] * ex  (col 64 is the gathered 1.0)
                # per bucket so the first aggregation matmuls start early
                ex_v = ex[:].rearrange("p (b t) -> p b t", b=4)
                for b in range(4):
                    nc.vector.tensor_tensor(
                        out=vslab[:, b, :, 0:65], in0=vslab[:, b, :, 0:65],
                        in1=ex_v[:, b, :, None].to_broadcast([P, GT, 65]),
                        op=Alu.mult)
                # aggregation (per window) + group-batched epilogue;
                # pure-padding tiles contribute zero and are skipped
                aggs = ep.tile([65, EPW], F32, tag="aggs")
                for wl in range(GRP):
                    w = g * GRP + wl
                    used = [(b, t) for b in range(4) for t in range(TPB)
                            if t < tiles_used[w][b]]
                    psagg = psC.tile([65, 128], F32)
                    for k, (b, t) in enumerate(used):
                        tcall = wl * TPB + t
                        nc.tensor.matmul(
                            psagg[:], lhsT=vslab[:, b, tcall, 0:65],
                            rhs=mslab[:, b, :, tcall],
                            start=(k == 0), stop=(k == len(used) - 1))
                    nc.scalar.copy(aggs[:, wl * 128:(wl + 1) * 128],
                                   psagg[:])
                den = ep.tile([1, EPW], F32, tag="den")
                nc.scalar.activation(den[:], aggs[64:65, :], Act.Copy,
                                     bias=EPS)
                nc.vector.reciprocal_approx_fast(den[:], den[:])
                ps_rec = psD.tile([D, EPW], F32)
                nc.tensor.matmul(ps_rec[:], lhsT=ones1[:], rhs=den[:],
                                 start=True, stop=True)
                tmp = ep.tile([D, EPW], F32, tag="tmp")
                nc.vector.tensor_tensor(out=tmp[:], in0=aggs[0:64, :],
                                        in1=ps_rec[:], op=Alu.mult)
                w0 = g * GRP
                if layer == 0:
                    nc.scalar.activation(x2T[:, w0 * P:(w0 + GRP) * P],
                                         tmp[:], Act.Relu, bias=bias[:, 0:1])
                    # layer-2 phase A, deferred one group so its x2T input
                    # is long done and PE never stalls at the queue head
                    if g >= 1:
                        wa = (g - 1) * GRP
                        phase_a_grp(1, wa, [x2T[:, (wa + k) * P:
                                                (wa + k + 1) * P]
                                            for k in range(GRP)])
                    if g == NG - 1:
                        wa = g * GRP
                        phase_a_grp(1, wa, [x2T[:, (wa + k) * P:
                                                (wa + k + 1) * P]
                                            for k in range(GRP)])
                else:
                    o2 = ep.tile([D, EPW], F32, tag="tmp2")
                    nc.scalar.activation(o2[:], tmp[:], Act.Identity,
                                         bias=bias[:, 0:1])
                    nc.sync.dma_start(
                        out_d.ap()[:, w0 * P:(w0 + GRP) * P], o2[:])

            def phase_b(layer):
                # All a_d expansion stt's run during the AllGather (they
                # only need adrep + dstloc), hiding the collective behind
                # real work.
                nc.gpsimd.collective_compute(
                    "AllGather", mybir.AluOpType.bypass,
                    replica_groups=[list(range(NCORES))],
                    ins=[shard[layer].ap()[:, :]],
                    outs=[tbl[layer].ap()[:, :]],
                )
                wb = 0.15 + layer * 1.6
                with tc.tile_wait_until(wb), tc.high_priority():
                    for g in range(NG):
                        c0 = g * NCOLS
                        for ci in range(NCOLS):
                            b, t = divmod(ci, GT)
                            w = g * GRP + t // TPB
                            if t % TPB >= tiles_used[w][b]:
                                continue  # column is pure padding
                            nc.vector.scalar_tensor_tensor(
                                out=scr_d[:],
                                in0=iota_b[:],
                                scalar=dstl_all[:, c0 + ci:c0 + ci + 1],
                                in1=adrep[:, w * 128:(w + 1) * 128],
                                op0=Alu.is_equal, op1=Alu.mult,
                                accum_out=adpe_d[:, c0 + ci:c0 + ci + 1])
                # steady loop, software-pipelined; wait-ts hints pin the
                # scheduler's per-engine order to the emission order
                ws = wb + 0.30
                dg = 0.013
                idxs = {0: load_idx(0), 1: load_idx(1)}
                with tc.tile_wait_until(ws):
                    st = head(layer, 0, idxs[0], dstl_all)
                for g in range(NG):
                    if g + 2 < NG:
                        with tc.tile_wait_until(ws + g * dg):
                            idxs[g + 2] = load_idx(g + 2)
                    if g + 1 < NG:
                        with tc.tile_wait_until(ws + (g + 1) * dg):
                            st_next = head(layer, g + 1, idxs.pop(g + 1),
                                           dstl_all)
                    with tc.tile_wait_until(ws + g * dg + 0.006):
                        tail(layer, g, st, dstl_all)
                    if g + 1 < NG:
                        st = st_next

            for layer in (0, 1):
                phase_b(layer)

    nc.compile()
    return nc


def kernel(x, edge_index, W1, att_src1, att_dst1, b1, W2, att_src2,
           att_dst2, b2):
    from concourse.bass_utils import run_bass_kernel_spmd

    x = np.asarray(x, np.float32)
    edge_index = np.asarray(edge_index)
    W1 = np.asarray(W1, np.float32)
    W2 = np.asarray(W2, np.float32)

    ek = edge_index.tobytes()
    cached = _CACHE.get("pre")
    if cached is not None and cached[0] == ek and \
            np.array_equal(cached[1], x):
        _, _, meta, percore, node_of_slot = cached
    else:
        meta, percore, node_of_slot = _preprocess(x, edge_index)
        _CACHE["pre"] = (ek, x.copy(), meta, percore, node_of_slot)
    mk = tuple(sorted(meta.items()))
    cached = _CACHE.get("prog")
    if cached is not None and cached[0] == mk:
        nc = cached[1]
    else:
        nc = _build_program(meta)
        _CACHE["prog"] = (mk, nc)
    SLOTS_PC, NSLOT, N = meta["SLOTS_PC"], meta["NSLOT"], meta["N"]

    w1cat = np.concatenate([W1, (W1 @ np.asarray(att_src1, np.float32))[:, None]],
                           axis=1).astype(BF16)
    w2cat = np.concatenate([W2, (W2 @ np.asarray(att_src2, np.float32))[:, None]],
                           axis=1).astype(BF16)
    wd1 = np.tile((W1 @ np.asarray(att_dst1, np.float32))[:, None],
                  (1, 128)).astype(BF16)
    wd2 = np.tile((W2 @ np.asarray(att_dst2, np.float32))[:, None],
                  (1, 128)).astype(BF16)
    b1c = np.asarray(b1, np.float32)[:, None]
    b2c = np.asarray(b2, np.float32)[:, None]
    ones1 = np.ones((1, D), np.float32)

    in_maps = []
    for c in range(NCORES):
        in_maps.append({
            "xT": percore["xT"][c], "gidx": percore["gidx"][c],
            "dstloc": percore["dstloc"][c],
            "w1cat": w1cat, "w2cat": w2cat, "wd1rep": wd1, "wd2rep": wd2,
            "b1": b1c, "b2": b2c, "ones1": ones1,
        })
    res = run_bass_kernel_spmd(nc, in_maps, list(range(NCORES)))

    out = np.empty((N, D), np.float32)
    for c in range(NCORES):
        blk = res.results[c]["out2T"]  # [64, SLOTS_PC]
        sl = node_of_slot[c * SLOTS_PC:(c + 1) * SLOTS_PC]
        valid = sl >= 0
        out[sl[valid]] = blk.T[valid]
    return out



# revision 16
# speedup vs baseline: 1.2083x; 1.1516x over previous
"""Two-layer single-head GAT on Trainium2 (8 NeuronCores, Bass/Tile).

Strategy (graph-parallel over dst nodes):
  - Relabel nodes into "slots": 8 cores x NW windows x 128 slots. Nodes are
    assigned to cores balanced by degree (serpentine over degree-sorted
    order), then packed into windows (<=128 nodes, capped total in-degree,
    capped per-src-bucket in-degree).
  - Per layer, each core builds its shard of a node table
    row[n] = [h(64) bf16 | 1.0 | a_s_hi | a_s_lo] via matmuls (h = x@W,
    a_s = x@(W@att_src)), then the 8 shards are AllGathered so every core
    holds the full table in its DRAM.
  - Edges live on the core that owns their dst. Per-edge rows h[src] are
    fetched with dma_gather (int16 idx => the table is read in 4 bucket
    slices of <=32768 rows; bucket(src) = src_core//2).
  - One-hot scatter masks M_T[e, d, t] = (d == dst_local[e,t]) are built on
    DVE with a repeated-iota constant so both operands are packed bf16
    (2x DVE mode). Per-edge a_d[dst] comes from a scalar_tensor_tensor
    against a replicated a_d row, split across DVE and Pool engines.
  - ex = exp(leaky_relu(a_s+a_d)); softmax max-subtraction is skipped
    (scores are O(10), exp stays in fp32 range; alpha is identical).
  - Aggregation: per window PSUM accumulates lhsT=[ex*h | ex] (128e x 65)
    @ rhs=M_T[:, :, t] (128e x 128d) -> [65 x 128d]; per-group epilogue
    divides by the ex-sum row, adds bias (and relu between layers).
  - Layer-2 phase A (table build) is interleaved into layer-1's edge phase
    per window to shorten the serial tail before the second AllGather.
Outputs are written transposed ([64, slots]) and un-permuted on the host.
"""

import numpy as np
import ml_dtypes

BF16 = ml_dtypes.bfloat16

NCORES = 8
P = 128
D = 64
NEG_SLOPE = 0.2
EPS = 1e-16

EWCAP = 2040      # max total in-degree per window
NODECAP = 128     # max nodes per window
TPBMAX = 5        # tiles per (window, bucket); bucket in-degree cap = 128*TPBMAX
GRP = 3           # windows per gather group (CALL=GRP*TPB*128 must stay
                  # under ~2500: one dma_gather's walrus sub-DMA semaphore
                  # arithmetic overflows a 16-bit ISA field beyond that)

_CACHE = {}


def _preprocess(x, edge_index):
    """Host-side partitioning/indexing. Returns per-core input arrays + meta."""
    N = x.shape[0]
    E = edge_index.shape[1]
    src = edge_index[0].astype(np.int64)
    dst = edge_index[1].astype(np.int64)

    deg = np.bincount(dst, minlength=N)

    # --- assign nodes to cores: serpentine over degree-sorted order ---
    order = np.argsort(-deg, kind="stable")
    core_of_node = np.empty(N, np.int32)
    pat = np.concatenate([np.arange(NCORES), np.arange(NCORES)[::-1]])
    core_of_node[order] = pat[np.arange(N) % (2 * NCORES)]

    bucket_of_node = core_of_node // 2  # 4 buckets of 2 cores each

    # per-node in-degree per src bucket
    deg_b = np.zeros((N, 4), np.int64)
    for b in range(4):
        m = bucket_of_node[src] == b
        deg_b[:, b] = np.bincount(dst[m], minlength=N)

    # --- pack windows per core ---
    bcap = P * TPBMAX
    windows = [[] for _ in range(NCORES)]  # list of lists of node ids
    for c in range(NCORES):
        nodes_c = order[core_of_node[order] == c]  # degree-sorted
        cur, cur_deg, cur_b = [], 0, np.zeros(4, np.int64)
        for n in nodes_c:
            d_n = deg[n]
            db_n = deg_b[n]
            if cur and (len(cur) >= NODECAP or cur_deg + d_n > EWCAP
                        or np.any(cur_b + db_n > bcap)):
                windows[c].append(cur)
                cur, cur_deg, cur_b = [], 0, np.zeros(4, np.int64)
            cur.append(n)
            cur_deg += d_n
            cur_b = cur_b + db_n
        if cur:
            windows[c].append(cur)

    nw_real = max(len(w) for w in windows)
    NG = -(-nw_real // GRP)
    NW = NG * GRP
    SLOTS_PC = NW * P
    NSLOT = NCORES * SLOTS_PC
    BSZ = NSLOT // 4
    assert BSZ <= 32768, f"int16 gather range exceeded: BSZ={BSZ}"

    # --- slot assignment ---
    slot_of_node = np.full(N, -1, np.int64)
    for c in range(NCORES):
        for w, wl in enumerate(windows[c]):
            base = c * SLOTS_PC + w * P
            slot_of_node[np.asarray(wl, np.int64)] = base + np.arange(len(wl))
    assert (slot_of_node >= 0).all()

    sslot = slot_of_node[src]
    dslot = slot_of_node[dst]
    ecore = (dslot // SLOTS_PC).astype(np.int32)
    ew = (dslot % SLOTS_PC) // P          # window within core
    eb = (sslot // BSZ).astype(np.int32)  # src bucket
    edloc = (dslot % P).astype(np.int32)  # dst slot within window
    esidx = (sslot % BSZ).astype(np.int64)  # idx within bucket slice

    # group edges by (core, window, bucket); order within a group is free
    key = ((ecore.astype(np.int64) * NW + ew) * 4 + eb)
    eorder = np.argsort(key, kind="stable")
    key_s = key[eorder]
    # counts per (c, w, b)
    cnt = np.bincount(key_s, minlength=NCORES * NW * 4).reshape(NCORES, NW, 4)
    tiles_used = -(-cnt.max(axis=0) // P)  # [NW, 4], same for all cores
    TPB = int(-(-cnt.max() // P))
    assert TPB <= TPBMAX, f"bucket cap violated: TPB={TPB}"
    CW = TPB * P                      # slots per (window, bucket)
    CALL = GRP * CW                   # idxs per dma_gather call
    NCOLS = 4 * GRP * TPB             # dstloc cols per group

    # per-core edge-slot tables
    gidx = np.zeros((NCORES, NG, 4, CALL), np.int16)
    dloc = np.full((NCORES, NG, 4, GRP * TPB, P), 300.0, np.float32)
    lastpos = np.zeros((NCORES, NG, 4), np.int64)

    starts = np.zeros(NCORES * NW * 4 + 1, np.int64)
    np.cumsum(np.bincount(key_s, minlength=NCORES * NW * 4), out=starts[1:])
    esidx_s = esidx[eorder]
    edloc_s = edloc[eorder]
    for c in range(NCORES):
        for w in range(NW):
            g, wl = divmod(w, GRP)
            for b in range(4):
                k = (c * NW + w) * 4 + b
                lo, hi = starts[k], starts[k + 1]
                n = hi - lo
                if n == 0:
                    continue
                off = wl * CW
                gidx[c, g, b, off:off + n] = esidx_s[lo:hi].astype(np.int16)
                tt = (np.arange(n) // P) + wl * TPB
                pp = np.arange(n) % P
                dloc[c, g, b, tt, pp] = edloc_s[lo:hi].astype(np.float32)
                lastpos[c, g, b] = max(lastpos[c, g, b], off + n)

    # wrap-16 + replicate to 128 partitions: [C, NG*4*128, CALL//16]
    g16 = gidx.reshape(NCORES, NG * 4, CALL // 16, 16).transpose(0, 1, 3, 2)
    g128 = np.tile(g16, (1, 1, 8, 1)).reshape(NCORES, NG * 4 * 128, CALL // 16)
    # dstloc: [C, 128, NG * 4*GRP*TPB]  col = g*NCOLS + b*(GRP*TPB) + t
    dl = dloc.transpose(0, 4, 1, 2, 3).reshape(NCORES, P, NG * NCOLS)
    dl = np.ascontiguousarray(dl).astype(BF16)

    # permuted, transposed x per core
    node_of_slot = np.full(NSLOT, -1, np.int64)
    node_of_slot[slot_of_node] = np.arange(N)
    xT = np.zeros((NCORES, D, SLOTS_PC), BF16)
    for c in range(NCORES):
        sl = node_of_slot[c * SLOTS_PC:(c + 1) * SLOTS_PC]
        valid = sl >= 0
        blk = np.zeros((SLOTS_PC, D), np.float32)
        blk[valid] = x[sl[valid]]
        xT[c] = blk.T.astype(BF16)

    meta = dict(NW=NW, NG=NG, TPB=TPB, CW=CW, CALL=CALL, NCOLS=NCOLS,
                SLOTS_PC=SLOTS_PC, NSLOT=NSLOT, BSZ=BSZ, N=N,
                tiles_used=tuple(map(tuple, tiles_used)))
    percore = dict(xT=xT, gidx=g128, dstloc=dl)
    return meta, percore, node_of_slot


def _build_program(meta):
    import concourse.bacc as bacc
    import concourse.tile as tile
    from concourse import mybir

    F32, BF, I16 = mybir.dt.float32, mybir.dt.bfloat16, mybir.dt.int16
    Alu = mybir.AluOpType
    Act = mybir.ActivationFunctionType

    NW, NG, TPB = meta["NW"], meta["NG"], meta["TPB"]
    tiles_used = meta["tiles_used"]
    CALL, NCOLS = meta["CALL"], meta["NCOLS"]
    SLOTS_PC, NSLOT, BSZ = meta["SLOTS_PC"], meta["NSLOT"], meta["BSZ"]
    GT = GRP * TPB
    EPW = 384  # epilogue batch width: GRP windows x 128 slots

    nc = bacc.Bacc("TRN2", target_bir_lowering=False, debug=False,
                   num_devices=NCORES)

    xT_d = nc.dram_tensor("xT", [D, SLOTS_PC], BF, kind="ExternalInput")
    gidx_d = nc.dram_tensor("gidx", [NG * 4 * 128, CALL // 16], I16,
                            kind="ExternalInput")
    dstloc_d = nc.dram_tensor("dstloc", [P, NG * NCOLS], BF,
                              kind="ExternalInput")
    w1cat_d = nc.dram_tensor("w1cat", [D, 65], BF, kind="ExternalInput")
    w2cat_d = nc.dram_tensor("w2cat", [D, 65], BF, kind="ExternalInput")
    wd1_d = nc.dram_tensor("wd1rep", [D, 128], BF, kind="ExternalInput")
    wd2_d = nc.dram_tensor("wd2rep", [D, 128], BF, kind="ExternalInput")
    b1_d = nc.dram_tensor("b1", [D, 1], F32, kind="ExternalInput")
    b2_d = nc.dram_tensor("b2", [D, 1], F32, kind="ExternalInput")
    ones1_d = nc.dram_tensor("ones1", [1, D], F32, kind="ExternalInput")
    out_d = nc.dram_tensor("out2T", [D, SLOTS_PC], F32, kind="ExternalOutput")

    shard = [nc.dram_tensor(f"shard{l}", [SLOTS_PC, 128], BF) for l in (1, 2)]
    tbl = [nc.dram_tensor(f"tbl{l}", [NSLOT, 128], BF, addr_space="Shared")
           for l in (1, 2)]

    with tile.TileContext(nc) as tc:
        import contextlib
        stack = contextlib.ExitStack()
        with stack:
            const = stack.enter_context(tc.tile_pool(name="const", bufs=1))
            small = stack.enter_context(tc.tile_pool(name="small", bufs=3))
            vp = stack.enter_context(tc.tile_pool(name="vp", bufs=3))
            mp = stack.enter_context(tc.tile_pool(name="mp", bufs=3))
            swp = stack.enter_context(tc.tile_pool(name="swp", bufs=2))
            msp = stack.enter_context(tc.tile_pool(name="msp", bufs=3))
            sc = stack.enter_context(tc.tile_pool(name="sc", bufs=3))
            ip = stack.enter_context(tc.tile_pool(name="ip", bufs=6))
            ep = stack.enter_context(tc.tile_pool(name="ep", bufs=2))
            psA = stack.enter_context(tc.tile_pool(name="psA", bufs=2, space="PSUM"))
            psC = stack.enter_context(tc.tile_pool(name="psC", bufs=2, space="PSUM"))
            psD = stack.enter_context(tc.tile_pool(name="psD", bufs=1, space="PSUM"))
            psT = stack.enter_context(tc.tile_pool(name="psT", bufs=2, space="PSUM"))
            psF = stack.enter_context(tc.tile_pool(name="psF", bufs=1, space="PSUM"))

            # constants
            iota_b = const.tile([P, 128], BF)
            iotar = const.tile([P, 128 * GT], BF)
            identity = const.tile([P, 128], BF)
            zero128 = const.tile([P, 128], BF)
            nc.gpsimd.memset(zero128[:], 0)
            with tc.tile_pool(name="iotatmp", bufs=1) as iotatmp:
                iota_i = iotatmp.tile([P, 128], I16)
                nc.gpsimd.iota(iota_i[:], pattern=[[1, 128]], base=0,
                               channel_multiplier=0)
                nc.vector.tensor_copy(iota_b[:], iota_i[:])
                # repeated iota: col = d*GT + t -> value d (for one-hot
                # builds with both operands packed => 2x DVE mode)
                iotar_i = iotatmp.tile([P, 128 * GT], I16)
                nc.gpsimd.iota(iotar_i[:], pattern=[[1, 128], [0, GT]],
                               base=0, channel_multiplier=0)
                nc.vector.tensor_copy(iotar[:], iotar_i[:])
                # per-partition iota column -> identity permutation matrix
                # (rhs of the PE transpose used in the a_d sweep)
                iotap_i = iotatmp.tile([P, 1], I16)
                nc.gpsimd.iota(iotap_i[:], pattern=[[0, 1]], base=0,
                               channel_multiplier=1)
                iotap = iotatmp.tile([P, 1], BF)
                nc.vector.tensor_copy(iotap[:], iotap_i[:])
                nc.vector.tensor_tensor(
                    out=identity[:],
                    in0=iotap[:].to_broadcast([P, 128]),
                    in1=iota_b[:], op=Alu.is_equal)
            ones1 = const.tile([1, D], F32)
            nc.sync.dma_start(ones1[:], ones1_d.ap()[:])
            w1cat = const.tile([D, 65], BF)
            nc.sync.dma_start(w1cat[:], w1cat_d.ap()[:])
            w2cat = const.tile([D, 65], BF)
            nc.sync.dma_start(w2cat[:], w2cat_d.ap()[:])
            wd1 = const.tile([D, 128], BF)
            nc.sync.dma_start(wd1[:], wd1_d.ap()[:])
            wd2 = const.tile([D, 128], BF)
            nc.sync.dma_start(wd2[:], wd2_d.ap()[:])
            b1 = const.tile([D, 1], F32)
            nc.sync.dma_start(b1[:], b1_d.ap()[:])
            b2 = const.tile([D, 1], F32)
            nc.sync.dma_start(b2[:], b2_d.ap()[:])

            # resident across layers
            x2T = const.tile([D, SLOTS_PC], BF)
            # a_d per dst slot, transposed: column w holds a_d of window w's
            # 128 slots along partitions (matvec rhs in the a_d sweep)
            adcolT = const.tile([P, NW], BF)
            adpe_d = const.tile([P, NG * NCOLS], F32)
            # whole-run dstloc table: shared by both layers, loaded once
            dstl_all = const.tile([P, NG * NCOLS], BF)
            nc.sync.dma_start(dstl_all[:], dstloc_d.ap()[:, :])

            for i in range(3):
                vs0 = vp.tile([P, 4, GT, 128], BF, tag="vslab")
                nc.gpsimd.memset(vs0[:], 0)

            # tblrow pool buffers get their constant-1 column (position 64)
            # written once; later phase-A writes never touch that column.
            tbl_tiles = []
            for i in range(3):
                tr = small.tile([P, GRP, 67], BF, tag="tblrow")
                nc.gpsimd.memset(tr[:, :, 64:65], 1.0)
                tbl_tiles.append(tr)

            def phase_a_grp(layer, w0, lhs_list):
                """Table rows + replicated-a_d for GRP consecutive windows,
                with one batched shard write. During the edge phase (layer 1
                interleaved) the copies run on Activation (idle there);
                standalone phase A balances them onto DVE."""
                wcat = w1cat if layer == 0 else w2cat
                wdcol = wd1 if layer == 0 else wd2
                tbuf = small.tile([P, GRP, 67], BF, tag="tblrow")
                psab = psA.tile([P, GRP, 66], F32)
                for k in range(GRP):
                    lhs = lhs_list[k]
                    nc.tensor.matmul(psab[:, k, 0:65], lhsT=lhs, rhs=wcat[:],
                                     start=True, stop=True)
                    # a_d of this window's slots, slots on partitions
                    nc.tensor.matmul(psab[:, k, 65:66], lhsT=lhs,
                                     rhs=wdcol[:, 0:1], start=True, stop=True)
                    if layer == 0:
                        nc.vector.tensor_copy(tbuf[:, k, 0:64],
                                              psab[:, k, 0:64])
                        nc.vector.tensor_copy(tbuf[:, k, 65:66],
                                              psab[:, k, 64:65])
                    else:
                        nc.scalar.copy(tbuf[:, k, 0:64], psab[:, k, 0:64])
                        nc.scalar.copy(tbuf[:, k, 65:66], psab[:, k, 64:65])
                    # a_s_lo residual for extra precision
                    nc.vector.tensor_tensor(out=tbuf[:, k, 66:67],
                                            in0=psab[:, k, 64:65],
                                            in1=tbuf[:, k, 65:66],
                                            op=Alu.subtract)
                if layer == 0:
                    nc.vector.tensor_copy(adcolT[:, w0:w0 + GRP],
                                          psab[:, :, 65])
                else:
                    nc.scalar.copy(adcolT[:, w0:w0 + GRP], psab[:, :, 65])
                nc.sync.dma_start(
                    shard[layer].ap()[w0 * P:(w0 + GRP) * P, 0:67]
                    .rearrange("(k p) c -> p k c", k=GRP),
                    tbuf[:])

            # ---- layer-0 phase A (x loaded in 12-window chunks) ----
            XC = 12
            for w0 in range(0, NW, XC):
                xtc = small.tile([D, XC * P], BF, tag="xtc")
                hi = min(NW, w0 + XC)
                nc.sync.dma_start(xtc[:, 0:(hi - w0) * P],
                                  xT_d.ap()[:, w0 * P:hi * P])
                for g0 in range(w0, hi, GRP):
                    phase_a_grp(0, g0, [
                        xtc[:, (g0 + k - w0) * P:(g0 + k - w0 + 1) * P]
                        for k in range(GRP)])

            iotar_v = iotar[:].rearrange("p (d t) -> p d t", t=GT)

            def load_idx(g):
                """Start DMA of the gather index table for group g."""
                idxt = ip.tile([128, 4, CALL // 16], I16, tag="idxt")
                r0 = g * 4 * 128
                nc.sync.dma_start(
                    idxt[:],
                    gidx_d.ap()[r0:r0 + 4 * 128, :]
                    .rearrange("(b p) c -> p b c", b=4))
                return idxt

            def head(layer, g, idxt, dstl_all):
                """Gather launch + one-hot build for group g."""
                tbl_l = tbl[layer].ap()
                vslab = vp.tile([P, 4, GT, 128], BF, tag="vslab")
                for b in range(4):
                    nc.gpsimd.dma_gather(
                        out_ap=vslab[:, b, :, :],
                        in_ap=tbl_l[b * BSZ:(b + 1) * BSZ, :],
                        idxs_ap=idxt[:, b, :], num_idxs=CALL,
                        num_idxs_reg=CALL,
                        elem_size=128, single_packet=False)
                # one-hot masks M_T[e, b, d, t] in one 2x-DVE instruction
                mslab = mp.tile([P, 4, 128, GT], BF, tag="mslab")
                dstl_v = dstl_all[:, g * NCOLS:(g + 1) * NCOLS] \
                    .rearrange("p (b t) -> p b t", b=4)
                nc.vector.tensor_tensor(
                    out=mslab[:],
                    in0=iotar_v[:, None, :, :].to_broadcast([P, 4, 128, GT]),
                    in1=dstl_v[:, :, None, :].to_broadcast([P, 4, 128, GT]),
                    op=Alu.is_equal)
                return vslab, mslab

            def tail(layer, g, st, dstl_all):
                """Score + aggregate + epilogue for group g (inputs ready)."""
                bias = b1 if layer == 0 else b2
                vslab, mslab = st
                c0 = g * NCOLS
                as_t = sc.tile([P, NCOLS], F32, tag="as_t")
                nc.vector.tensor_tensor(
                    out=as_t[:].rearrange("p (b t) -> p b t", b=4),
                    in0=vslab[:, :, :, 65], in1=vslab[:, :, :, 66],
                    op=Alu.add)
                nc.vector.tensor_tensor(out=as_t[:],
                                        in0=as_t[:],
                                        in1=adpe_d[:, c0:c0 + NCOLS],
                                        op=Alu.add)
                lr = sc.tile([P, NCOLS], F32, tag="lr")
                nc.vector.scalar_tensor_tensor(
                    out=lr[:], in0=as_t[:], scalar=NEG_SLOPE,
                    in1=as_t[:], op0=Alu.mult, op1=Alu.max)
                ex = sc.tile([P, NCOLS], F32, tag="ex")
                nc.scalar.activation(ex[:], lr[:], Act.Exp)
                # V' in place: [h|1] * ex  (col 64 is the gathered 1.0)
                # per bucket so the first aggregation matmuls start early
                ex_v = ex[:].rearrange("p (b t) -> p b t", b=4)
                for b in range(4):
                    nc.vector.tensor_tensor(
                        out=vslab[:, b, :, 0:65], in0=vslab[:, b, :, 0:65],
                        in1=ex_v[:, b, :, None].to_broadcast([P, GT, 65]),
                        op=Alu.mult)
                # aggregation (per window) + group-batched epilogue;
                # pure-padding tiles contribute zero and are skipped
                aggs = ep.tile([65, EPW], F32, tag="aggs")
                for wl in range(GRP):
                    w = g * GRP + wl
                    used = [(b, t) for b in range(4) for t in range(TPB)
                            if t < tiles_used[w][b]]
                    psagg = psC.tile([65, 128], F32)
                    if not used:  # window with no edges: zero the bank
                        nc.tensor.matmul(psagg[:], lhsT=zero128[:, 0:65],
                                         rhs=zero128[:],
                                         start=True, stop=True)
                    for k, (b, t) in enumerate(used):
                        tcall = wl * TPB + t
                        nc.tensor.matmul(
                            psagg[:], lhsT=vslab[:, b, tcall, 0:65],
                            rhs=mslab[:, b, :, tcall],
                            start=(k == 0), stop=(k == len(used) - 1))
                    nc.scalar.copy(aggs[:, wl * 128:(wl + 1) * 128],
                                   psagg[:])
                den = ep.tile([1, EPW], F32, tag="den")
                nc.scalar.activation(den[:], aggs[64:65, :], Act.Copy,
                                     bias=EPS)
                nc.vector.reciprocal_approx_fast(den[:], den[:])
                ps_rec = psD.tile([D, EPW], F32)
                nc.tensor.matmul(ps_rec[:], lhsT=ones1[:], rhs=den[:],
                                 start=True, stop=True)
                tmp = ep.tile([D, EPW], F32, tag="tmp")
                nc.vector.tensor_tensor(out=tmp[:], in0=aggs[0:64, :],
                                        in1=ps_rec[:], op=Alu.mult)
                w0 = g * GRP
                if layer == 0:
                    nc.scalar.activation(x2T[:, w0 * P:(w0 + GRP) * P],
                                         tmp[:], Act.Relu, bias=bias[:, 0:1])
                    # layer-2 phase A, deferred one group so its x2T input
                    # is long done and PE never stalls at the queue head
                    if g >= 1:
                        wa = (g - 1) * GRP
                        phase_a_grp(1, wa, [x2T[:, (wa + k) * P:
                                                (wa + k + 1) * P]
                                            for k in range(GRP)])
                    if g == NG - 1:
                        wa = g * GRP
                        phase_a_grp(1, wa, [x2T[:, (wa + k) * P:
                                                (wa + k + 1) * P]
                                            for k in range(GRP)])
                else:
                    o2 = ep.tile([D, EPW], F32, tag="tmp2")
                    nc.scalar.activation(o2[:], tmp[:], Act.Identity,
                                         bias=bias[:, 0:1])
                    nc.sync.dma_start(
                        out_d.ap()[:, w0 * P:(w0 + GRP) * P], o2[:])

            def phase_b(layer):
                # The a_d-expansion sweep runs during the AllGather
                # (it only needs adcolT + dstloc), hiding the collective
                # behind real work.
                nc.gpsimd.collective_compute(
                    "AllGather", mybir.AluOpType.bypass,
                    replica_groups=[list(range(NCORES))],
                    ins=[shard[layer].ap()[:, :]],
                    outs=[tbl[layer].ap()[:, :]],
                )
                # a_d expansion sweep, timed to fill the AllGather window:
                # per group, build the one-hot masks once on DVE, transpose
                # used tiles on PE (via the identity permutation), copy the
                # transposed masks to SBUF (DVE/Act alternating), then one
                # near-free PE matvec per tile gives a_d[dst] per edge.
                wb = 0.15 + layer * 1.6
                with tc.tile_wait_until(wb), tc.high_priority():
                    for g in range(NG):
                        c0 = g * NCOLS
                        msw = swp.tile([P, 4, 128, GT], BF, tag="msw")
                        dstl_v = dstl_all[:, c0:c0 + NCOLS] \
                            .rearrange("p (b t) -> p b t", b=4)
                        nc.vector.tensor_tensor(
                            out=msw[:],
                            in0=iotar_v[:, None, :, :]
                            .to_broadcast([P, 4, 128, GT]),
                            in1=dstl_v[:, :, None, :]
                            .to_broadcast([P, 4, 128, GT]),
                            op=Alu.is_equal)
                        psf = psF.tile([P, NCOLS], F32)
                        cols = []
                        for ci in range(NCOLS):
                            b, t = divmod(ci, GT)
                            w = g * GRP + t // TPB
                            cols.append((ci, b, t, w,
                                         t % TPB < tiles_used[w][b]))
                        usedc = [c for c in cols if c[4]]
                        for j0 in range(0, len(usedc), 4):
                            ch = usedc[j0:j0 + 4]
                            pst = psT.tile([P, 512], BF)
                            for j, (ci, b, t, w, _) in enumerate(ch):
                                nc.tensor.transpose(
                                    pst[:, j * 128:(j + 1) * 128],
                                    msw[:, b, :, t], identity[:])
                            mss = msp.tile([P, 512], BF, tag="msS")
                            wid = len(ch) * 128
                            if (j0 // 4) % 2 == 0:
                                nc.vector.tensor_copy(mss[:, 0:wid],
                                                      pst[:, 0:wid])
                            else:
                                nc.scalar.copy(mss[:, 0:wid], pst[:, 0:wid])
                            for j, (ci, b, t, w, _) in enumerate(ch):
                                nc.tensor.matmul(
                                    psf[:, ci:ci + 1],
                                    lhsT=mss[:, j * 128:(j + 1) * 128],
                                    rhs=adcolT[:, w:w + 1],
                                    start=True, stop=True)
                        for (ci, b, t, w, u) in cols:
                            if not u:  # keep padding columns finite (zero)
                                nc.tensor.matmul(
                                    psf[:, ci:ci + 1], lhsT=zero128[:],
                                    rhs=adcolT[:, w:w + 1],
                                    start=True, stop=True)
                        nc.vector.tensor_copy(adpe_d[:, c0:c0 + NCOLS],
                                              psf[:])
                # steady loop, software-pipelined; wait-ts hints pin the
                # scheduler's per-engine order to the emission order
                ws = wb + 0.30
                dg = 0.013
                idxs = {0: load_idx(0), 1: load_idx(1)}
                with tc.tile_wait_until(ws):
                    st = head(layer, 0, idxs[0], dstl_all)
                for g in range(NG):
                    if g + 2 < NG:
                        with tc.tile_wait_until(ws + g * dg):
                            idxs[g + 2] = load_idx(g + 2)
                    if g + 1 < NG:
                        with tc.tile_wait_until(ws + (g + 1) * dg):
                            st_next = head(layer, g + 1, idxs.pop(g + 1),
                                           dstl_all)
                    with tc.tile_wait_until(ws + g * dg + 0.006):
                        tail(layer, g, st, dstl_all)
                    if g + 1 < NG:
                        st = st_next

            for layer in (0, 1):
                phase_b(layer)

    nc.compile()
    return nc


def kernel(x, edge_index, W1, att_src1, att_dst1, b1, W2, att_src2,
           att_dst2, b2):
    from concourse.bass_utils import run_bass_kernel_spmd

    x = np.asarray(x, np.float32)
    edge_index = np.asarray(edge_index)
    W1 = np.asarray(W1, np.float32)
    W2 = np.asarray(W2, np.float32)

    ek = edge_index.tobytes()
    cached = _CACHE.get("pre")
    if cached is not None and cached[0] == ek and \
            np.array_equal(cached[1], x):
        _, _, meta, percore, node_of_slot = cached
    else:
        meta, percore, node_of_slot = _preprocess(x, edge_index)
        _CACHE["pre"] = (ek, x.copy(), meta, percore, node_of_slot)
    mk = tuple(sorted(meta.items()))
    cached = _CACHE.get("prog")
    if cached is not None and cached[0] == mk:
        nc = cached[1]
    else:
        nc = _build_program(meta)
        _CACHE["prog"] = (mk, nc)
    SLOTS_PC, NSLOT, N = meta["SLOTS_PC"], meta["NSLOT"], meta["N"]

    w1cat = np.concatenate([W1, (W1 @ np.asarray(att_src1, np.float32))[:, None]],
                           axis=1).astype(BF16)
    w2cat = np.concatenate([W2, (W2 @ np.asarray(att_src2, np.float32))[:, None]],
                           axis=1).astype(BF16)
    wd1 = np.tile((W1 @ np.asarray(att_dst1, np.float32))[:, None],
                  (1, 128)).astype(BF16)
    wd2 = np.tile((W2 @ np.asarray(att_dst2, np.float32))[:, None],
                  (1, 128)).astype(BF16)
    b1c = np.asarray(b1, np.float32)[:, None]
    b2c = np.asarray(b2, np.float32)[:, None]
    ones1 = np.ones((1, D), np.float32)

    in_maps = []
    for c in range(NCORES):
        in_maps.append({
            "xT": percore["xT"][c], "gidx": percore["gidx"][c],
            "dstloc": percore["dstloc"][c],
            "w1cat": w1cat, "w2cat": w2cat, "wd1rep": wd1, "wd2rep": wd2,
            "b1": b1c, "b2": b2c, "ones1": ones1,
        })
    res = run_bass_kernel_spmd(nc, in_maps, list(range(NCORES)))

    out = np.empty((N, D), np.float32)
    for c in range(NCORES):
        blk = res.results[c]["out2T"]  # [64, SLOTS_PC]
        sl = node_of_slot[c * SLOTS_PC:(c + 1) * SLOTS_PC]
        valid = sl >= 0
        out[sl[valid]] = blk.T[valid]
    return out

